# revision 20
# baseline (speedup 1.0000x reference)
"""Multi-head attention Trainium2 kernel (8 NeuronCores, SPMD).

Problem: B=4, N=2048, E=1024, H=16, d_k=64, fp32 I/O.

Sharding: 8 cores = (batch b, query-half). Each core gets x[b] rolled so
its 1024 queries are tokens 0..1023 (attention is permutation-equivariant
over keys, so rolling keys/values is harmless). K/V projections are
duplicated between the two cores of a batch (cheaper than any collective).

Per-core dataflow (bf16 matmul operands, fp32 PSUM accumulation):
  x^T  via bf16 DMA-transpose                  [E, N] feature-major
  Q^T = Wq^T x^T   (lhsT=Wq, rhs=x^T)          [E, NQ]
  K^T = Wk^T x^T                               [E, N]
  V   = x Wv       (lhsT=x^T, rhs=Wv)          [N, E], stored head-packed
                                               with a ones column per head
  S^T = K Q^T per head (row-packed pairs, contraction d_k=64)
  W^T = exp(S^T/8)  (no max subtraction; scores are in [-2.2, 2.2])
  attnT_h = [V_h|1]^T W^T_h  accumulated over k in PSUM -> row 64 = softmax
            denominators (ride along free in the same matmul)
  ATT[h*64+d, q] = attnT_h[d, q] / denom_h[q]
  out = ATT^T Wo + bo

Schedule notes (perf):
  - consecutive matmuls share the stationary operand wherever possible so
    the PE reuses LDWEIGHTS (HW-measured -60us on the projection phase)
  - the attention units (whose pace is set by the ACT exp stream) start
    right after V + one Q/K subtile; the remaining Q/K subtiles and the
    first output-projection chunks are emitted as low-priority "filler"
    inside later units, so the PE slack under the ACT-bound phase absorbs
    them instead of delaying the exp stream start
  - the output projection is fed per-128-query chunk as soon as its att
    columns are normalized; the remainder runs in the tail
"""

import numpy as np
import ml_dtypes

import concourse.bass as bass
import concourse.mybir as mybir
from concourse import bacc
from concourse.tile import TileContext
from concourse import bass_utils

BF16 = mybir.dt.bfloat16
F32 = mybir.dt.float32
F32R = mybir.dt.float32r
AF = mybir.ActivationFunctionType

N = 2048      # tokens per batch (keys)
NQ = 1024     # queries per core
E = 1024      # embed dim
H = 16        # heads
D = 64        # head dim
P = 128
EO = E // P   # 8 E-subtiles
NKC = N // P  # 16 key chunks of 128
NG = 4        # key groups (of 512 tokens) for K/V chunked tiles
QH = 512      # query sub-block for attention units
NPAIR = H // 2


def build_mha_kernel(repeat: int = 1, stop_after: str | None = None):
    """repeat>1 wraps the whole body in an on-device loop (timing builds).
    stop_after='proj' builds a truncated kernel (timing experiments only)."""
    nc = bacc.Bacc("TRN2", target_bir_lowering=False, debug=False, num_devices=8)

    x_bf = nc.dram_tensor("x_bf", [N, E], BF16, kind="ExternalInput")
    wq_d = nc.dram_tensor("wq_bf", [E, E], BF16, kind="ExternalInput")
    wk_d = nc.dram_tensor("wk_bf", [E, E], BF16, kind="ExternalInput")
    wv_d = nc.dram_tensor("wv_bf", [E, E], BF16, kind="ExternalInput")
    wo_d = nc.dram_tensor("wo_bf", [E, E], BF16, kind="ExternalInput")
    bq_d = nc.dram_tensor("bq", [E], F32, kind="ExternalInput")
    bk_d = nc.dram_tensor("bk", [E], F32, kind="ExternalInput")
    bv_d = nc.dram_tensor("bv", [E], F32, kind="ExternalInput")
    bo_d = nc.dram_tensor("bo", [E], F32, kind="ExternalInput")
    out_d = nc.dram_tensor("out", [NQ, E], F32, kind="ExternalOutput")

    out_v = out_d.ap().rearrange("(qo p) f -> p qo f", p=P)  # [128, 8, 1024]

    from contextlib import ExitStack
    with TileContext(nc) as tc, ExitStack() as _loop:
        if repeat > 1:
            _loop.enter_context(tc.For_i(0, repeat, 1))
        with (
            tc.tile_pool(name="const", bufs=1) as const,
            tc.tile_pool(name="wstream", bufs=2) as wstream,
            tc.tile_pool(name="qt", bufs=1) as qt_pool,
            tc.tile_pool(name="kt", bufs=1) as kt_pool,
            tc.tile_pool(name="vt", bufs=1) as vt_pool,
            tc.tile_pool(name="wt", bufs=4) as wt_pool,
            tc.tile_pool(name="outs", bufs=2) as out_pool,
            tc.tile_pool(name="norm", bufs=1) as norm_pool,
            tc.tile_pool(name="xt", bufs=1) as xt_pool,
            tc.tile_pool(name="att", bufs=1) as att_pool,
            tc.tile_pool(name="acc_ps", bufs=2, space="PSUM") as acc_ps,
            tc.tile_pool(name="wide_ps", bufs=3, space="PSUM") as wide_ps,
        ):
            # ---- constants ----
            bqp = const.tile([P, EO], F32)  # per-partition bias for Q^T
            nc.sync.dma_start(bqp[:], bq_d.ap().rearrange("(o p) -> p o", p=P))
            bkp = const.tile([P, EO], F32)
            nc.sync.dma_start(bkp[:], bk_d.ap().rearrange("(o p) -> p o", p=P))
            bv1 = const.tile([1, E], F32)
            nc.sync.dma_start(bv1[:], bv_d.ap().rearrange("(one f) -> one f", one=1))
            bo1 = const.tile([1, E], F32)
            nc.sync.dma_start(bo1[:], bo_d.ap().rearrange("(one f) -> one f", one=1))
            ones_f = const.tile([D + 1, P], F32)
            nc.vector.memset(ones_f[:], 1.0)
            ones1f = const.tile([1, P], F32R)
            nc.vector.tensor_copy(ones1f[:], ones_f[:1, :])
            ones65 = const.tile([D + 1, D], F32R)  # row 64 used as lhsT for bcast
            nc.vector.tensor_copy(ones65[:], ones_f[:, :D])

            # broadcast per-free biases onto all 128 partitions via matmul
            def bcast128(src1, name):
                srcr = const.tile([1, E], F32R, tag=f"bcr_{name}")
                nc.vector.tensor_copy(srcr[:], src1[:])
                dst = const.tile([P, E], BF16, tag=f"bc_{name}")
                for c in range(E // 512):
                    ps = acc_ps.tile([P, 512], F32, tag="acc")
                    nc.tensor.matmul(
                        ps[:], ones1f[:, :P],
                        srcr[:, c * 512:(c + 1) * 512],
                        start=True, stop=True,
                    )
                    nc.vector.tensor_copy(dst[:, c * 512:(c + 1) * 512], ps[:])
                return dst

            bvb = bcast128(bv1, "bv")   # [128, 1024] V bias replicated
            bob = bcast128(bo1, "bo")   # [128, 1024] out bias replicated

            # ---- x^T via DMA transpose (bf16) ----
            xt = xt_pool.tile([P, EO, N], BF16)  # x^T: [E-part, E-sub, tok]
            for o in range(EO):
                nc.sync.dma_start_transpose(
                    xt[:, o, :], x_bf.ap()[:, o * P:(o + 1) * P]
                )

            # ---- weight streams (2 slots; wv's readers all finish in the
            # prologue, so wk recycles its slot without a cycle) ----
            wv = wstream.tile([P, EO, E], BF16, tag="w", name="wv")
            for o in range(EO):
                nc.sync.dma_start(wv[:, o, :], wv_d.ap()[o * P:(o + 1) * P, :])
            wq = wstream.tile([P, EO, E], BF16, tag="w", name="wq")
            for o in range(EO):
                nc.sync.dma_start(wq[:, o, :], wq_d.ap()[o * P:(o + 1) * P, :])

            qt = qt_pool.tile([P, EO, NQ], BF16)
            kts = [kt_pool.tile([P, EO, 512], BF16, tag=f"kt{g}", name=f"kt{g}")
                   for g in range(NG)]
            vts = [vt_pool.tile([P, 4, H * (D + 1)], BF16, tag=f"vt{g}",
                                name=f"vt{g}")
                   for g in range(NG)]

            def q_proj(o):
                # both query chunks share the stationary wq subtile (LDW reuse)
                pw = wide_ps.tile([P, 2 * QH], F32, tag="wide", name="qpw")
                pss = [pw[:, qc * 512:(qc + 1) * 512] for qc in range(NQ // 512)]
                for k in range(EO):
                    for qc, ps in enumerate(pss):
                        nc.tensor.matmul(
                            ps, wq[:, k, o * P:(o + 1) * P],
                            xt[:, k, qc * 512:(qc + 1) * 512],
                            start=(k == 0), stop=(k == EO - 1),
                        )
                for qc, ps in enumerate(pss):
                    nc.vector.tensor_scalar_add(
                        qt[:, o, qc * 512:(qc + 1) * 512], ps, bqp[:, o:o + 1]
                    )

            def k_proj(o, gpair):
                # the key-group pair shares the stationary wk subtile
                gs = (2 * gpair, 2 * gpair + 1)
                pw = wide_ps.tile([P, 2 * QH], F32, tag="wide", name="kpw")
                pss = [pw[:, i * 512:(i + 1) * 512] for i in range(2)]
                for k in range(EO):
                    for g, ps in zip(gs, pss):
                        nc.tensor.matmul(
                            ps, wk[:, k, o * P:(o + 1) * P],
                            xt[:, k, g * 512:(g + 1) * 512],
                            start=(k == 0), stop=(k == EO - 1),
                        )
                for g, ps in zip(gs, pss):
                    nc.vector.tensor_scalar_add(
                        kts[g][:, o, :], ps, bkp[:, o:o + 1]
                    )

            def v_proj(g, fc):
                # feature chunk fc (heads 8*fc..8*fc+7) for all 4 token
                # chunks of group g
                vtg = vts[g]
                vh = vtg.rearrange("p t (h c) -> p t h c", c=D + 1)
                if fc == 0:
                    nc.vector.memset(vh[:, :, :, D:D + 1], 1.0)
                for t in range(4):
                    tok = g * 4 + t
                    ps = acc_ps.tile([P, 512], F32, tag="acc", name="v")
                    for k in range(EO):
                        nc.tensor.matmul(
                            ps[:], xt[:, k, tok * P:(tok + 1) * P],
                            wv[:, k, fc * 512:(fc + 1) * 512],
                            start=(k == 0), stop=(k == EO - 1),
                        )
                    # scatter [128, 512] -> 8 heads x 64 cols (stride 65)
                    nc.vector.tensor_tensor(
                        vh[:, t, fc * 8:(fc + 1) * 8, :D],
                        ps[:].rearrange("p (h c) -> p h c", c=D),
                        bvb[:, fc * 512:(fc + 1) * 512]
                        .rearrange("p (h c) -> p h c", c=D),
                        mybir.AluOpType.add,
                    )

            # V fully up front (every attention unit sweeps all key groups
            # within ~18us, so vt must not lag), plus the first Q/K subtile.
            # wk DMA issues after wv (wstream has 2 slots; wk reuses wq's
            # slot only after Q finishes, so K weights stream separately).
            v_proj(0, 0)
            q_proj(0)
            for g in range(NG):
                for fc in range(2):
                    if (g, fc) != (0, 0):
                        v_proj(g, fc)
            wk = wstream.tile([P, EO, E], BF16, tag="w", name="wk")
            for o in range(EO):
                nc.sync.dma_start(wk[:, o, :], wk_d.ap()[o * P:(o + 1) * P, :])
            k_proj(0, 0)
            k_proj(0, 1)

            if stop_after == "proj":
                for o in range(1, EO):
                    q_proj(o)
                    k_proj(o, 0)
                    k_proj(o, 1)
                for qc in range(2):
                    ot = out_pool.tile([P, 512], F32, tag="out")
                    nc.vector.tensor_copy(ot[:], qt[:, 0, qc * 512:(qc + 1) * 512])
                    nc.sync.dma_start(out_v[:, qc, :512], ot[:])
                nc.compile()
                return nc

            # ---- attention units, with filler work interleaved ----
            att = att_pool.tile([P, EO, NQ], BF16)  # attnT, head-pair packed

            def make_norm(j, qs, apA, apB):
                def norm():
                    rs = norm_pool.tile([D + 1, 2 * QH], F32, tag="rsum")
                    nc.vector.tensor_copy(rs[D:D + 1, :QH], apA[D:D + 1, :])
                    nc.vector.tensor_copy(rs[D:D + 1, QH:], apB[D:D + 1, :])
                    rc = norm_pool.tile([D + 1, 2 * QH], F32R, tag="rcp")
                    with nc.allow_low_precision(reason="f32r recip bcast"):
                        nc.vector.reciprocal(rc[D:D + 1, :], rs[D:D + 1, :])
                    rb = norm_pool.tile([D, 2 * QH], F32, tag="rb")
                    for half, st0 in ((0, 0), (1, QH)):
                        rbp = wide_ps.tile([P, 2 * QH], F32, tag="wide")
                        nc.tensor.matmul(
                            rbp[:D, :QH],
                            ones65[D:D + 1, :],
                            rc[D:D + 1, st0:st0 + QH],
                            start=True, stop=True,
                        )
                        nc.vector.tensor_copy(rb[:, st0:st0 + QH], rbp[:D, :QH])
                    nc.vector.tensor_tensor(
                        att[:D, j, qs], apA[:D, :], rb[:, :QH],
                        mybir.AluOpType.mult,
                    )
                    tmb = norm_pool.tile([D, QH], BF16, tag="tmb")
                    nc.vector.tensor_tensor(
                        tmb[:], apB[:D, :], rb[:, QH:], mybir.AluOpType.mult
                    )
                    nc.sync.dma_start(att[D:P, j, qs], tmb[:])
                return norm

            wo = None

            def o_proj(qc):
                q0 = qc * P
                # both feature chunks share the stationary att subtile
                pss = [wide_ps.tile([P, 2 * QH], F32, tag="wide",
                                    name=f"ps_o{fc}") for fc in range(2)]
                for o in range(EO):
                    for fc, ps_w in enumerate(pss):
                        nc.tensor.matmul(
                            ps_w[:, :512],
                            att[:, o, q0:q0 + P],
                            wo[:, o, fc * 512:(fc + 1) * 512],
                            start=(o == 0), stop=(o == EO - 1),
                        )
                for fc, ps_w in enumerate(pss):
                    ot = out_pool.tile([P, 512], F32, tag="out")
                    nc.vector.tensor_tensor(
                        ot[:], ps_w[:, :512], bob[:, fc * 512:(fc + 1) * 512],
                        mybir.AluOpType.add,
                    )
                    nc.sync.dma_start(
                        out_v[:, qc, fc * 512:(fc + 1) * 512], ot[:]
                    )

            state = {"pending_norm": None, "filler": []}

            def emit_unit(qh, j):
                qs = slice(qh * QH, (qh + 1) * QH)
                ha, hb = 2 * j, 2 * j + 1
                apA_t = acc_ps.tile([P, QH], F32, tag="acc", name="apA")
                apB_t = acc_ps.tile([P, QH], F32, tag="acc", name="apB")
                apA, apB = apA_t[:D + 1, :], apB_t[:D + 1, :]
                # software-pipelined: S^T/exp run one kc ahead of AV;
                # previous pair's normalize is deferred into this loop
                wt_chunks = {}

                def emit_st_exp(kc):
                    g, col = kc // 4, (kc % 4) * P
                    ktg = kts[g]
                    st2 = wide_ps.tile([P, 2 * QH], F32, tag="wide")
                    for i, h in enumerate((ha, hb)):
                        lo = (h % 2) * D
                        nc.tensor.matmul(
                            st2[:, i * QH:(i + 1) * QH],
                            ktg[lo:lo + D, h // 2, col:col + P],
                            qt[lo:lo + D, h // 2, qs],
                            start=True, stop=True,
                        )
                    wt2 = wt_pool.tile([P, 2 * QH], BF16, tag="wt")
                    nc.scalar.activation(wt2[:], st2[:], AF.Exp, scale=0.125)
                    wt_chunks[kc] = wt2

                def emit_av(kc):
                    g = kc // 4
                    vtg = vts[g]
                    wt2 = wt_chunks.pop(kc)
                    for i, (h, ap_out) in enumerate(((ha, apA), (hb, apB))):
                        nc.tensor.matmul(
                            ap_out[:],
                            vtg[:, kc % 4, h * (D + 1):(h + 1) * (D + 1)],
                            wt2[:, i * QH:(i + 1) * QH],
                            start=(kc == 0), stop=(kc == NKC - 1),
                        )

                emit_st_exp(0)
                for kc in range(1, NKC):
                    emit_st_exp(kc)
                    if kc == 6:
                        # projection/output filler rides along while ACT is
                        # the bottleneck; at kc==6 the deferred norm has
                        # already freed its PSUM accumulator slots, so the
                        # filler's accumulators don't stall the PE
                        for f in state["filler"]:
                            f()
                        state["filler"] = []
                    emit_av(kc - 1)
                emit_av(NKC - 1)
                make_norm(j, qs, apA, apB)()

            # fillers: unit (0, j) carries the Q/K subtile j+1 needed by the
            # NEXT unit; the second query half's units carry the first four
            # output-projection chunks (qh0 columns of att are final then).
            for j in range(NPAIR):
                # unit (0,j) needs subtile j (made in unit (0,j-1)'s filler,
                # or the prologue for j=0); it carries subtile j+1
                state["filler"] = ([lambda o=j + 1: q_proj(o),
                                    lambda o=j + 1: k_proj(o, 0),
                                    lambda o=j + 1: k_proj(o, 1)]
                                   if j + 1 < EO else [])
                emit_unit(0, j)
            # wo stream lands while qh1 units run
            wo = wstream.tile([P, EO, E], BF16, tag="w", name="wo")
            for o in range(EO):
                nc.sync.dma_start(wo[:, o, :], wo_d.ap()[o * P:(o + 1) * P, :])
            for j in range(NPAIR):
                state["filler"] = [lambda qc=j - 2: o_proj(qc)] if 2 <= j < 6 else []
                emit_unit(1, j)
            # ---- remaining output-projection chunks ----
            for qc in range(4, NQ // P):
                o_proj(qc)

    nc.compile()
    return nc


_NC_CACHE = None


def kernel(x, W_q, b_q, W_k, b_k, W_v, b_v, W_o, b_o):
    global _NC_CACHE
    if _NC_CACHE is None:
        _NC_CACHE = build_mha_kernel()
    nc = _NC_CACHE

    bf = ml_dtypes.bfloat16
    x = np.asarray(x, np.float32)
    shared = {
        "wq_bf": np.asarray(W_q, np.float32).astype(bf),
        "wk_bf": np.asarray(W_k, np.float32).astype(bf),
        "wv_bf": np.asarray(W_v, np.float32).astype(bf),
        "wo_bf": np.asarray(W_o, np.float32).astype(bf),
        "bq": np.asarray(b_q, np.float32),
        "bk": np.asarray(b_k, np.float32),
        "bv": np.asarray(b_v, np.float32),
        "bo": np.asarray(b_o, np.float32),
    }

    in_maps = []
    for c in range(8):
        b, half = c // 2, c % 2
        xb = x[b]
        if half:
            xb = np.roll(xb, -NQ, axis=0)
        in_maps.append({"x_bf": np.ascontiguousarray(xb.astype(bf)), **shared})

    res = bass_utils.run_bass_kernel_spmd(nc, in_maps, core_ids=list(range(8)))

    out = np.empty((4, N, E), np.float32)
    for c in range(8):
        b, half = c // 2, c % 2
        out[b, half * NQ:(half + 1) * NQ] = res.results[c]["out"]
    return out


# revision 24
# speedup vs baseline: 1.0524x; 1.0524x over previous
"""Multi-head attention Trainium2 kernel (8 NeuronCores, SPMD).

Problem: B=4, N=2048, E=1024, H=16, d_k=64, fp32 I/O.

Sharding: 8 cores = (batch b, query-half). Each core gets x[b] rolled so
its 1024 queries are tokens 0..1023 (attention is permutation-equivariant
over keys, so rolling keys/values is harmless). K/V projections are
duplicated between the two cores of a batch (cheaper than any collective).

Per-core dataflow (bf16 matmul operands, fp32 PSUM accumulation):
  x^T  via bf16 DMA-transpose                  [E, N] feature-major
  Q^T = Wq^T x^T   (lhsT=Wq, rhs=x^T)          [E, NQ]
  K^T = Wk^T x^T                               [E, N]
  V   = x Wv       (lhsT=x^T, rhs=Wv)          [N, E], stored head-packed
                                               with a ones column per head
  S^T = K Q^T per head (row-packed pairs, contraction d_k=64)
  W^T = exp(S^T/8)  (no max subtraction; scores are in [-2.2, 2.2])
  attnT_h = [V_h|1]^T W^T_h  accumulated over k in PSUM -> row 64 = softmax
            denominators (ride along free in the same matmul)
  ATT[h*64+d, q] = attnT_h[d, q] / denom_h[q]
  out = ATT^T Wo + bo

Schedule notes (perf):
  - consecutive matmuls share the stationary operand wherever possible so
    the PE reuses LDWEIGHTS (HW-measured -60us on the projection phase)
  - the attention units (whose pace is set by the ACT exp stream) start
    right after V + one Q/K subtile; the remaining Q/K subtiles and the
    first output-projection chunks are emitted as low-priority "filler"
    inside later units, so the PE slack under the ACT-bound phase absorbs
    them instead of delaying the exp stream start
  - the output projection is fed per-128-query chunk as soon as its att
    columns are normalized; the remainder runs in the tail
"""

import numpy as np
import ml_dtypes

import concourse.bass as bass
import concourse.mybir as mybir
from concourse import bacc
from concourse.tile import TileContext
from concourse import bass_utils

BF16 = mybir.dt.bfloat16
F32 = mybir.dt.float32
F32R = mybir.dt.float32r
AF = mybir.ActivationFunctionType

N = 2048      # tokens per batch (keys)
NQ = 1024     # queries per core
E = 1024      # embed dim
H = 16        # heads
D = 64        # head dim
P = 128
EO = E // P   # 8 E-subtiles
NKC = N // P  # 16 key chunks of 128
NG = 4        # key groups (of 512 tokens) for K/V chunked tiles
QH = 512      # query sub-block for attention units
NPAIR = H // 2


def build_mha_kernel(repeat: int = 1, stop_after: str | None = None):
    """repeat>1 wraps the whole body in an on-device loop (timing builds).
    stop_after='proj' builds a truncated kernel (timing experiments only)."""
    nc = bacc.Bacc("TRN2", target_bir_lowering=False, debug=False, num_devices=8)

    x_bf = nc.dram_tensor("x_bf", [N, E], BF16, kind="ExternalInput")
    wq_d = nc.dram_tensor("wq_bf", [E, E], BF16, kind="ExternalInput")
    wk_d = nc.dram_tensor("wk_bf", [E, E], BF16, kind="ExternalInput")
    wv_d = nc.dram_tensor("wv_bf", [E, E], BF16, kind="ExternalInput")
    wo_d = nc.dram_tensor("wo_bf", [E, E], BF16, kind="ExternalInput")
    bq_d = nc.dram_tensor("bq", [E], F32, kind="ExternalInput")
    bk_d = nc.dram_tensor("bk", [E], F32, kind="ExternalInput")
    bv_d = nc.dram_tensor("bv", [E], F32, kind="ExternalInput")
    bo_d = nc.dram_tensor("bo", [E], F32, kind="ExternalInput")
    out_d = nc.dram_tensor("out", [NQ, E], F32, kind="ExternalOutput")

    out_v = out_d.ap().rearrange("(qo p) f -> p qo f", p=P)  # [128, 8, 1024]

    from contextlib import ExitStack
    with TileContext(nc) as tc, ExitStack() as _loop:
        if repeat > 1:
            _loop.enter_context(tc.For_i(0, repeat, 1))
        with (
            tc.tile_pool(name="const", bufs=1) as const,
            tc.tile_pool(name="wstream", bufs=2) as wstream,
            tc.tile_pool(name="qt", bufs=1) as qt_pool,
            tc.tile_pool(name="kt", bufs=1) as kt_pool,
            tc.tile_pool(name="vt", bufs=1) as vt_pool,
            tc.tile_pool(name="wt", bufs=3) as wt_pool,
            tc.tile_pool(name="outs", bufs=2) as out_pool,
            tc.tile_pool(name="norm", bufs=1) as norm_pool,
            tc.tile_pool(name="xt", bufs=1) as xt_pool,
            tc.tile_pool(name="att", bufs=1) as att_pool,
            tc.tile_pool(name="acc_ps", bufs=2, space="PSUM") as acc_ps,
            tc.tile_pool(name="wide_ps", bufs=3, space="PSUM") as wide_ps,
        ):
            # ---- constants ----
            bqp = const.tile([P, EO], F32)  # per-partition bias for Q^T
            nc.sync.dma_start(bqp[:], bq_d.ap().rearrange("(o p) -> p o", p=P))
            bkp = const.tile([P, EO], F32)
            nc.sync.dma_start(bkp[:], bk_d.ap().rearrange("(o p) -> p o", p=P))
            bv1 = const.tile([1, E], F32)
            nc.sync.dma_start(bv1[:], bv_d.ap().rearrange("(one f) -> one f", one=1))
            bo1 = const.tile([1, E], F32)
            nc.sync.dma_start(bo1[:], bo_d.ap().rearrange("(one f) -> one f", one=1))
            ones_f = const.tile([D + 1, P], F32)
            nc.vector.memset(ones_f[:], 1.0)
            ones1f = const.tile([1, P], F32R)
            nc.vector.tensor_copy(ones1f[:], ones_f[:1, :])
            ones65 = const.tile([D + 1, D], F32R)  # row 64 used as lhsT for bcast
            nc.vector.tensor_copy(ones65[:], ones_f[:, :D])

            # broadcast per-free biases onto all 128 partitions via matmul
            def bcast128(src1, name):
                srcr = const.tile([1, E], F32R, tag=f"bcr_{name}")
                nc.vector.tensor_copy(srcr[:], src1[:])
                dst = const.tile([P, E], BF16, tag=f"bc_{name}")
                for c in range(E // 512):
                    ps = acc_ps.tile([P, 512], F32, tag="acc")
                    nc.tensor.matmul(
                        ps[:], ones1f[:, :P],
                        srcr[:, c * 512:(c + 1) * 512],
                        start=True, stop=True,
                    )
                    nc.vector.tensor_copy(dst[:, c * 512:(c + 1) * 512], ps[:])
                return dst

            bvb = bcast128(bv1, "bv")   # [128, 1024] V bias replicated
            bob = bcast128(bo1, "bo")   # [128, 1024] out bias replicated

            # ---- x^T via DMA transpose (bf16) ----
            xt = xt_pool.tile([P, EO, N], BF16)  # x^T: [E-part, E-sub, tok]
            for o in range(EO):
                nc.sync.dma_start_transpose(
                    xt[:, o, :], x_bf.ap()[:, o * P:(o + 1) * P]
                )

            # ---- weight streams (2 slots; wv's readers all finish in the
            # prologue, so wk recycles its slot without a cycle) ----
            wv = wstream.tile([P, EO, E], BF16, tag="w", name="wv")
            for o in range(EO):
                nc.sync.dma_start(wv[:, o, :], wv_d.ap()[o * P:(o + 1) * P, :])
            wq = wstream.tile([P, EO, E], BF16, tag="w", name="wq")
            for o in range(EO):
                nc.sync.dma_start(wq[:, o, :], wq_d.ap()[o * P:(o + 1) * P, :])

            qt = qt_pool.tile([P, EO, NQ], BF16)
            kts = [kt_pool.tile([P, EO, 512], BF16, tag=f"kt{g}", name=f"kt{g}")
                   for g in range(NG)]
            vts = [vt_pool.tile([P, 4, H * (D + 1)], BF16, tag=f"vt{g}",
                                name=f"vt{g}")
                   for g in range(NG)]

            def q_proj(o):
                # both query chunks share the stationary wq subtile (LDW reuse)
                pw = wide_ps.tile([P, 2 * QH], F32, tag="wide", name="qpw")
                pss = [pw[:, qc * 512:(qc + 1) * 512] for qc in range(NQ // 512)]
                for k in range(EO):
                    for qc, ps in enumerate(pss):
                        nc.tensor.matmul(
                            ps, wq[:, k, o * P:(o + 1) * P],
                            xt[:, k, qc * 512:(qc + 1) * 512],
                            start=(k == 0), stop=(k == EO - 1),
                        )
                for qc, ps in enumerate(pss):
                    nc.vector.tensor_scalar_add(
                        qt[:, o, qc * 512:(qc + 1) * 512], ps, bqp[:, o:o + 1]
                    )

            def k_proj(o, gpair):
                # the key-group pair shares the stationary wk subtile
                gs = (2 * gpair, 2 * gpair + 1)
                pw = wide_ps.tile([P, 2 * QH], F32, tag="wide", name="kpw")
                pss = [pw[:, i * 512:(i + 1) * 512] for i in range(2)]
                for k in range(EO):
                    for g, ps in zip(gs, pss):
                        nc.tensor.matmul(
                            ps, wk[:, k, o * P:(o + 1) * P],
                            xt[:, k, g * 512:(g + 1) * 512],
                            start=(k == 0), stop=(k == EO - 1),
                        )
                for g, ps in zip(gs, pss):
                    nc.vector.tensor_scalar_add(
                        kts[g][:, o, :], ps, bkp[:, o:o + 1]
                    )

            def v_proj(g, fc):
                # feature chunk fc (heads 8*fc..8*fc+7) for all 4 token
                # chunks of group g
                vtg = vts[g]
                vh = vtg.rearrange("p t (h c) -> p t h c", c=D + 1)
                if fc == 0:
                    nc.vector.memset(vh[:, :, :, D:D + 1], 1.0)
                for t in range(4):
                    tok = g * 4 + t
                    ps = acc_ps.tile([P, 512], F32, tag="acc", name="v")
                    for k in range(EO):
                        nc.tensor.matmul(
                            ps[:], xt[:, k, tok * P:(tok + 1) * P],
                            wv[:, k, fc * 512:(fc + 1) * 512],
                            start=(k == 0), stop=(k == EO - 1),
                        )
                    # scatter [128, 512] -> 8 heads x 64 cols (stride 65)
                    nc.vector.tensor_tensor(
                        vh[:, t, fc * 8:(fc + 1) * 8, :D],
                        ps[:].rearrange("p (h c) -> p h c", c=D),
                        bvb[:, fc * 512:(fc + 1) * 512]
                        .rearrange("p (h c) -> p h c", c=D),
                        mybir.AluOpType.add,
                    )

            # V fully up front (every attention unit sweeps all key groups
            # within ~18us, so vt must not lag), plus the first Q/K subtile.
            # wk DMA issues after wv (wstream has 2 slots; wk reuses wq's
            # slot only after Q finishes, so K weights stream separately).
            v_proj(0, 0)
            q_proj(0)
            for g in range(NG):
                for fc in range(2):
                    if (g, fc) != (0, 0):
                        v_proj(g, fc)
            wk = wstream.tile([P, EO, E], BF16, tag="w", name="wk")
            for o in range(EO):
                nc.sync.dma_start(wk[:, o, :], wk_d.ap()[o * P:(o + 1) * P, :])
            k_proj(0, 0)
            k_proj(0, 1)

            if stop_after == "proj":
                for o in range(1, EO):
                    q_proj(o)
                    k_proj(o, 0)
                    k_proj(o, 1)
                for qc in range(2):
                    ot = out_pool.tile([P, 512], F32, tag="out")
                    nc.vector.tensor_copy(ot[:], qt[:, 0, qc * 512:(qc + 1) * 512])
                    nc.sync.dma_start(out_v[:, qc, :512], ot[:])
                nc.compile()
                return nc

            # ---- attention units, with filler work interleaved ----
            att = att_pool.tile([P, EO, NQ], BF16)  # attnT, head-pair packed

            def evac_norm(j, qs, apA, apB):
                """Evacuate the PSUM accumulators NOW (frees both acc banks
                within ~2us of the last AV matmul, so the next unit's AV can
                start), and return the rest of the normalize as a deferred
                closure over the SBUF copies.  Denominators stay f32; the
                attn values are bf16 anyway once normalized."""
                rs = norm_pool.tile([D + 1, 2 * QH], F32, tag="rsum")
                nc.vector.tensor_copy(rs[D:D + 1, :QH], apA[D:D + 1, :])
                nc.vector.tensor_copy(rs[D:D + 1, QH:], apB[D:D + 1, :])
                evA = norm_pool.tile([D, QH], BF16, tag="evA")
                nc.vector.tensor_copy(evA[:], apA[:D, :])
                evB = norm_pool.tile([D, QH], BF16, tag="evB")
                nc.vector.tensor_copy(evB[:], apB[:D, :])

                def norm():
                    rc = norm_pool.tile([D + 1, 2 * QH], F32R, tag="rcp")
                    with nc.allow_low_precision(reason="f32r recip bcast"):
                        nc.vector.reciprocal(rc[D:D + 1, :], rs[D:D + 1, :])
                    rb = norm_pool.tile([D, 2 * QH], F32, tag="rb")
                    for half, st0 in ((0, 0), (1, QH)):
                        rbp = wide_ps.tile([P, 2 * QH], F32, tag="wide")
                        nc.tensor.matmul(
                            rbp[:D, :QH],
                            ones65[D:D + 1, :],
                            rc[D:D + 1, st0:st0 + QH],
                            start=True, stop=True,
                        )
                        nc.vector.tensor_copy(rb[:, st0:st0 + QH], rbp[:D, :QH])
                    nc.vector.tensor_tensor(
                        att[:D, j, qs], evA[:], rb[:, :QH],
                        mybir.AluOpType.mult,
                    )
                    nc.vector.tensor_tensor(
                        evB[:], evB[:], rb[:, QH:], mybir.AluOpType.mult
                    )
                    nc.sync.dma_start(att[D:P, j, qs], evB[:])
                return norm

            wo = None

            def o_proj(qc):
                q0 = qc * P
                # both feature chunks share the stationary att subtile
                pss = [wide_ps.tile([P, 2 * QH], F32, tag="wide",
                                    name=f"ps_o{fc}") for fc in range(2)]
                for o in range(EO):
                    for fc, ps_w in enumerate(pss):
                        nc.tensor.matmul(
                            ps_w[:, :512],
                            att[:, o, q0:q0 + P],
                            wo[:, o, fc * 512:(fc + 1) * 512],
                            start=(o == 0), stop=(o == EO - 1),
                        )
                for fc, ps_w in enumerate(pss):
                    ot = out_pool.tile([P, 512], F32, tag="out")
                    nc.vector.tensor_tensor(
                        ot[:], ps_w[:, :512], bob[:, fc * 512:(fc + 1) * 512],
                        mybir.AluOpType.add,
                    )
                    nc.sync.dma_start(
                        out_v[:, qc, fc * 512:(fc + 1) * 512], ot[:]
                    )

            state = {"pending_norm": None, "filler": []}

            def emit_unit(qh, j):
                qs = slice(qh * QH, (qh + 1) * QH)
                ha, hb = 2 * j, 2 * j + 1
                apA_t = acc_ps.tile([P, QH], F32, tag="acc", name="apA")
                apB_t = acc_ps.tile([P, QH], F32, tag="acc", name="apB")
                apA, apB = apA_t[:D + 1, :], apB_t[:D + 1, :]
                # software-pipelined: S^T/exp run one kc ahead of AV;
                # previous pair's normalize is deferred into this loop
                wt_chunks = {}

                def emit_st_exp(kc):
                    g, col = kc // 4, (kc % 4) * P
                    ktg = kts[g]
                    st2 = wide_ps.tile([P, 2 * QH], F32, tag="wide")
                    for i, h in enumerate((ha, hb)):
                        lo = (h % 2) * D
                        nc.tensor.matmul(
                            st2[:, i * QH:(i + 1) * QH],
                            ktg[lo:lo + D, h // 2, col:col + P],
                            qt[lo:lo + D, h // 2, qs],
                            start=True, stop=True,
                        )
                    wt2 = wt_pool.tile([P, 2 * QH], BF16, tag="wt")
                    nc.scalar.activation(wt2[:], st2[:], AF.Exp, scale=0.125)
                    wt_chunks[kc] = wt2

                def emit_av(kc):
                    g = kc // 4
                    vtg = vts[g]
                    wt2 = wt_chunks.pop(kc)
                    for i, (h, ap_out) in enumerate(((ha, apA), (hb, apB))):
                        nc.tensor.matmul(
                            ap_out[:],
                            vtg[:, kc % 4, h * (D + 1):(h + 1) * (D + 1)],
                            wt2[:, i * QH:(i + 1) * QH],
                            start=(kc == 0), stop=(kc == NKC - 1),
                        )

                emit_st_exp(0)
                for kc in range(1, NKC):
                    emit_st_exp(kc)
                    if kc == 2 and state["pending_norm"] is not None:
                        # previous pair's recip/broadcast/mult, deferred on
                        # its SBUF evacuation copies
                        state["pending_norm"]()
                        state["pending_norm"] = None
                    if kc == 6:
                        # projection/output filler rides along while ACT is
                        # the bottleneck of this phase
                        for f in state["filler"]:
                            f()
                        state["filler"] = []
                    emit_av(kc - 1)
                emit_av(NKC - 1)
                state["pending_norm"] = evac_norm(j, qs, apA, apB)

            # fillers: unit (0, j) carries the Q/K subtile j+1 needed by the
            # NEXT unit; the second query half's units carry the first four
            # output-projection chunks (qh0 columns of att are final then).
            for j in range(NPAIR):
                # unit (0,j) needs subtile j (made in unit (0,j-1)'s filler,
                # or the prologue for j=0); it carries subtile j+1
                state["filler"] = ([lambda o=j + 1: q_proj(o),
                                    lambda o=j + 1: k_proj(o, 0),
                                    lambda o=j + 1: k_proj(o, 1)]
                                   if j + 1 < EO else [])
                emit_unit(0, j)
            # wo stream lands while qh1 units run
            wo = wstream.tile([P, EO, E], BF16, tag="w", name="wo")
            for o in range(EO):
                nc.sync.dma_start(wo[:, o, :], wo_d.ap()[o * P:(o + 1) * P, :])
            for j in range(NPAIR):
                state["filler"] = [lambda qc=j - 2: o_proj(qc)] if 2 <= j < 6 else []
                emit_unit(1, j)
            if state["pending_norm"] is not None:
                state["pending_norm"]()
                state["pending_norm"] = None
            # ---- remaining output-projection chunks ----
            for qc in range(4, NQ // P):
                o_proj(qc)

    nc.compile()
    return nc


_NC_CACHE = None


def kernel(x, W_q, b_q, W_k, b_k, W_v, b_v, W_o, b_o):
    global _NC_CACHE
    if _NC_CACHE is None:
        _NC_CACHE = build_mha_kernel()
    nc = _NC_CACHE

    bf = ml_dtypes.bfloat16
    x = np.asarray(x, np.float32)
    shared = {
        "wq_bf": np.asarray(W_q, np.float32).astype(bf),
        "wk_bf": np.asarray(W_k, np.float32).astype(bf),
        "wv_bf": np.asarray(W_v, np.float32).astype(bf),
        "wo_bf": np.asarray(W_o, np.float32).astype(bf),
        "bq": np.asarray(b_q, np.float32),
        "bk": np.asarray(b_k, np.float32),
        "bv": np.asarray(b_v, np.float32),
        "bo": np.asarray(b_o, np.float32),
    }

    in_maps = []
    for c in range(8):
        b, half = c // 2, c % 2
        xb = x[b]
        if half:
            xb = np.roll(xb, -NQ, axis=0)
        in_maps.append({"x_bf": np.ascontiguousarray(xb.astype(bf)), **shared})

    res = bass_utils.run_bass_kernel_spmd(nc, in_maps, core_ids=list(range(8)))

    out = np.empty((4, N, E), np.float32)
    for c in range(8):
        b, half = c // 2, c % 2
        out[b, half * NQ:(half + 1) * NQ] = res.results[c]["out"]
    return out
